# revision 2
# baseline (speedup 1.0000x reference)
"""Trainium2 Bass kernel for InterpretableMultiHeadAttention.

Reference computation (B=4, S=1024, D=1024, H=8, DK=128):
    qs = einsum('bsd,hdk->hbsk', q, Wq); ks = einsum('btd,hdk->hbtk', k, Wk)
    vs = einsum('btd,dk->btk', v, Wv)
    attn = softmax(qs @ ks^T / sqrt(DK))            # [H,B,S,S]
    out  = mean_h(attn @ vs) @ Wo                   # [B,S,D]
    returns (out, attn)

Sharding: 8 cores = 4 batches x 2 head-groups (4 heads each). Each core
computes its batch's 4 heads end to end plus a partial output projection
(its heads' share of the mean). Host sums the two partials per batch and
stacks/transposes attn.

On-device layout trick: scores are computed TRANSPOSED ([t,s], t on
partitions) so that the softmax denominator is a ones-vector matmul
(partition reduction on PE), the normalization is a broadcast multiply,
and the AV + output projections need no big transposes. attn is written
to DRAM as [h,t,s]; the host transposes views when assembling [H,B,S,S].

All matmul operands use float32r (full-rate fp32 on the PE at free-dim
>= 256, ~1.5e-4 relative error).
"""

import numpy as np

B, S, D, H, DK = 4, 1024, 1024, 8, 128
P = 128          # partitions
DO = D // P      # 8 contraction tiles
SC = S // 512    # 2 free-dim chunks of 512
HPC = H // 2     # 4 heads per core
N5 = 512

_CACHE = {}
TRACE = False
LAST_RESULTS = None


def _build():
    import concourse.bass as bass
    import concourse.tile as tile
    from concourse import bacc, mybir
    from concourse.masks import make_identity

    F32 = mybir.dt.float32
    F32R = mybir.dt.float32r
    Exp = mybir.ActivationFunctionType.Exp

    nc = bacc.Bacc("TRN2", target_bir_lowering=False, debug=False, num_devices=8)

    qt = nc.dram_tensor("qt", [P, DO, S], F32R, kind="ExternalInput")
    kt = nc.dram_tensor("kt", [P, DO, S], F32R, kind="ExternalInput")
    vt = nc.dram_tensor("vt", [P, DO, S], F32R, kind="ExternalInput")
    wq = nc.dram_tensor("wq", [P, HPC, DO, DK], F32R, kind="ExternalInput")
    wk = nc.dram_tensor("wk", [P, HPC, DO, DK], F32R, kind="ExternalInput")
    wv = nc.dram_tensor("wv", [P, DO, DK], F32R, kind="ExternalInput")
    wo = nc.dram_tensor("wo", [DK, D], F32R, kind="ExternalInput")
    ones = nc.dram_tensor("ones", [P, 1], F32R, kind="ExternalInput")

    attn_t = nc.dram_tensor("attn_t", [HPC, S, S], F32, kind="ExternalOutput")
    out_p = nc.dram_tensor("out_p", [S, D], F32, kind="ExternalOutput")

    with tile.TileContext(nc) as tc:
        with (
            tc.tile_pool(name="inp", bufs=1) as inp,
            tc.tile_pool(name="big", bufs=20) as big,
            tc.tile_pool(name="proj", bufs=2) as proj,
            tc.tile_pool(name="misc", bufs=1) as misc,
            tc.tile_pool(name="rbcp", bufs=2) as rbcp,
            tc.tile_pool(name="outp", bufs=3) as outp,
            tc.tile_pool(name="ps_mm", bufs=2, space="PSUM") as ps_mm,
            tc.tile_pool(name="ps_sc", bufs=2, space="PSUM") as ps_sc,
            tc.tile_pool(name="ps_av", bufs=2, space="PSUM") as ps_av,
            tc.tile_pool(name="ps_dn", bufs=2, space="PSUM") as ps_dn,
        ):
            qt_sb = inp.tile([P, DO, S], F32R)
            kt_sb = inp.tile([P, DO, S], F32R)
            wq_sb = inp.tile([P, HPC, DO, DK], F32R)
            wk_sb = inp.tile([P, HPC, DO, DK], F32R)
            wv_sb = inp.tile([P, DO, DK], F32R)
            wo_sb = inp.tile([DK, D], F32R)
            ones_sb = inp.tile([P, 1], F32R)
            ident = inp.tile([P, P], F32)
            nc.sync.dma_start(qt_sb[:], qt[:])
            nc.sync.dma_start(kt_sb[:], kt[:])
            nc.sync.dma_start(wq_sb[:], wq[:])
            nc.sync.dma_start(wk_sb[:], wk[:])
            nc.sync.dma_start(wv_sb[:], wv[:])
            nc.sync.dma_start(wo_sb[:], wo[:])
            nc.sync.dma_start(ones_sb[:], ones[:])
            make_identity(nc, ident[:])

            # ---- V projection: vsT[k,t] = Wv^T @ v^T, then PE-transpose to vs[t,k]
            vt_ch = {}
            for do in range(DO):
                for c2 in range(SC):
                    t = big.tile([P, N5], F32R, tag="big")
                    nc.sync.dma_start(t[:], vt[:, do, c2 * N5:(c2 + 1) * N5])
                    vt_ch[do, c2] = t
            vsT_sb = misc.tile([P, S], F32)  # [k, t]
            for c2 in range(SC):
                vps = ps_mm.tile([P, N5], F32, tag="ps_mm")
                for do in range(DO):
                    nc.tensor.matmul(vps[:], wv_sb[:, do, :], vt_ch[do, c2][:],
                                     start=(do == 0), stop=(do == DO - 1))
                nc.vector.tensor_copy(vsT_sb[:, c2 * N5:(c2 + 1) * N5], vps[:])
            vs_sb = misc.tile([P, DO, DK], F32R)  # [t, t_outer, k]
            for tt in range(DO):
                trp = ps_sc.tile([P, P], F32, tag="ps_sc")
                nc.tensor.transpose(trp[:], vsT_sb[:, tt * P:(tt + 1) * P], ident[:])
                nc.vector.tensor_copy(vs_sb[:, tt, :], trp[:])

            # ---- head loop
            avp = [ps_av.tile([P, N5], F32, tag="ps_av", name=f"avp{c2}")
                   for c2 in range(SC)]
            for h in range(HPC):
                qsT = proj.tile([P, S], F32R, tag="qsT")  # [k, s]
                ksT = proj.tile([P, S], F32R, tag="ksT")  # [k, t]
                for w_sb, src_sb, dst in ((wq_sb, qt_sb, qsT), (wk_sb, kt_sb, ksT)):
                    for c2 in range(SC):
                        pps = ps_mm.tile([P, N5], F32, tag="ps_mm")
                        for do in range(DO):
                            nc.tensor.matmul(
                                pps[:], w_sb[:, h, do, :],
                                src_sb[:, do, c2 * N5:(c2 + 1) * N5],
                                start=(do == 0), stop=(do == DO - 1))
                        nc.vector.tensor_copy(dst[:, c2 * N5:(c2 + 1) * N5], pps[:])

                # scoresT + exp: expT[t,s] = exp(ksT^T @ qsT)
                expT = {}
                for tt in range(DO):
                    for c2 in range(SC):
                        sps = ps_sc.tile([P, N5], F32, tag="ps_sc")
                        nc.tensor.matmul(sps[:], ksT[:, tt * P:(tt + 1) * P],
                                         qsT[:, c2 * N5:(c2 + 1) * N5],
                                         start=True, stop=True)
                        e = big.tile([P, N5], F32R, tag="big")
                        nc.scalar.activation(out=e[:], in_=sps[:], func=Exp)
                        expT[tt, c2] = e

                # softmax denominator over t (partition dim) via ones-matmul
                recip = misc.tile([1, S], F32, tag="recip")
                for c2 in range(SC):
                    dps = ps_dn.tile([1, N5], F32, tag="ps_dn")
                    for tt in range(DO):
                        nc.tensor.matmul(dps[:], ones_sb[:], expT[tt, c2][:],
                                         start=(tt == 0), stop=(tt == DO - 1))
                    nc.vector.reciprocal(recip[:, c2 * N5:(c2 + 1) * N5], dps[:])
                rbc = rbcp.tile([P, S], F32, tag="rbc")
                nc.gpsimd.partition_broadcast(rbc[:], recip[:])

                # normalize, write attn, accumulate AV over (t, h) into PSUM
                for tt in range(DO):
                    for c2 in range(SC):
                        a = big.tile([P, N5], F32R, tag="big")
                        nc.vector.tensor_mul(a[:], expT[tt, c2][:],
                                             rbc[:, c2 * N5:(c2 + 1) * N5])
                        nc.sync.dma_start(
                            attn_t[h, tt * P:(tt + 1) * P, c2 * N5:(c2 + 1) * N5],
                            a[:].bitcast(F32))
                        nc.tensor.matmul(avp[c2][:], vs_sb[:, tt, :], a[:],
                                         start=(h == 0 and tt == 0),
                                         stop=(h == HPC - 1 and tt == DO - 1),
                                         skip_group_check=True)

            # ---- output projection: out[s,m] = pooledT^T @ (Wo/H)
            pooledT = misc.tile([P, S], F32R)  # [k, s]
            for c2 in range(SC):
                nc.vector.tensor_copy(pooledT[:, c2 * N5:(c2 + 1) * N5], avp[c2][:])
            for st in range(DO):
                for mc in range(SC):
                    ops = ps_mm.tile([P, N5], F32, tag="ps_mm")
                    nc.tensor.matmul(ops[:], pooledT[:, st * P:(st + 1) * P],
                                     wo_sb[:, mc * N5:(mc + 1) * N5],
                                     start=True, stop=True)
                    ot = outp.tile([P, N5], F32, tag="ot")
                    nc.vector.tensor_copy(ot[:], ops[:])
                    nc.sync.dma_start(
                        out_p[st * P:(st + 1) * P, mc * N5:(mc + 1) * N5], ot[:])

    nc.compile()
    return nc


def _pack_inputs(q, k, v, Wq, Wk, Wv, Wo):
    """Build the 8 per-core input maps (host-side shard + layout packing)."""
    f32 = np.float32
    scale = np.float32(DK ** -0.5)
    # [p, h, do, k] = W[h, do*128+p, k]
    wq_g, wk_g = [], []
    for g in range(2):
        wq_g.append(np.ascontiguousarray(
            (Wq[g * HPC:(g + 1) * HPC] * scale).astype(f32)
            .reshape(HPC, DO, P, DK).transpose(2, 0, 1, 3)))
        wk_g.append(np.ascontiguousarray(
            Wk[g * HPC:(g + 1) * HPC].astype(f32)
            .reshape(HPC, DO, P, DK).transpose(2, 0, 1, 3)))
    wv_pack = np.ascontiguousarray(
        Wv.astype(f32).reshape(DO, P, DK).transpose(1, 0, 2))
    wo_pack = np.ascontiguousarray((Wo.astype(f32) / np.float32(H)))
    ones = np.ones([P, 1], f32)

    def pack_t(x):  # [S, D] -> [p, do, s] with d = do*128+p
        return np.ascontiguousarray(
            x.T.reshape(DO, P, S).transpose(1, 0, 2))

    in_maps = []
    for c in range(8):
        b, g = divmod(c, 2)
        in_maps.append({
            "qt": pack_t(np.asarray(q[b], f32)),
            "kt": pack_t(np.asarray(k[b], f32)),
            "vt": pack_t(np.asarray(v[b], f32)),
            "wq": wq_g[g], "wk": wk_g[g], "wv": wv_pack, "wo": wo_pack,
            "ones": ones,
        })
    return in_maps


def kernel(q, k, v, Wq, Wk, Wv, Wo):
    global LAST_RESULTS
    from concourse.bass_utils import run_bass_kernel_spmd

    if "nc" not in _CACHE:
        _CACHE["nc"] = _build()
    nc = _CACHE["nc"]

    in_maps = _pack_inputs(q, k, v, Wq, Wk, Wv, Wo)
    res = run_bass_kernel_spmd(nc, in_maps, core_ids=list(range(8)), trace=TRACE)
    LAST_RESULTS = res

    attn = np.empty((H, B, S, S), np.float32)
    outputs = np.empty((B, S, D), np.float32)
    for c in range(8):
        b, g = divmod(c, 2)
        attn[g * HPC:(g + 1) * HPC, b] = res.results[c]["attn_t"].transpose(0, 2, 1)
    for b in range(B):
        outputs[b] = res.results[2 * b]["out_p"] + res.results[2 * b + 1]["out_p"]
    return outputs, attn


# revision 9
# speedup vs baseline: 41160.0703x; 41160.0703x over previous
"""Trainium2 Bass kernel for InterpretableMultiHeadAttention.

Reference computation (B=4, S=1024, D=1024, H=8, DK=128):
    qs = einsum('bsd,hdk->hbsk', q, Wq); ks = einsum('btd,hdk->hbtk', k, Wk)
    vs = einsum('btd,dk->btk', v, Wv)
    attn = softmax(qs @ ks^T / sqrt(DK))            # [H,B,S,S]
    out  = mean_h(attn @ vs) @ Wo                   # [B,S,D]
    returns (out, attn)

Sharding: 8 cores = 4 batches x 2 head-groups (4 heads each). Each core
computes its batch's 4 heads end to end plus a partial output projection
(its heads' share of the mean). Host sums the two partials per batch and
stacks/transposes attn.

On-device layout trick: scores are computed TRANSPOSED ([t,s], t on
partitions) so that the softmax denominator is a ones-vector matmul
(partition reduction on PE), the normalization is a broadcast multiply,
and the AV + output projections need no big transposes. attn is written
to DRAM as [h,t,s]; the host transposes views when assembling [H,B,S,S].

All matmul operands use float32r (full-rate fp32 on the PE at free-dim
>= 256, ~1.5e-4 relative error; verified on hardware).
"""

import numpy as np

B, S, D, H, DK = 4, 1024, 1024, 8, 128
P = 128          # partitions
DO = D // P      # 8 contraction tiles
SC = S // 512    # 2 free-dim chunks of 512
HPC = H // 2     # 4 heads per core
N5 = 512

_CACHE = {}
TRACE = False
LAST_RESULTS = None


def _build():
    import concourse.bass as bass  # noqa: F401  (env sanity)
    import concourse.tile as tile
    from concourse import bacc, mybir
    from concourse.masks import make_identity

    F32 = mybir.dt.float32
    F32R = mybir.dt.float32r
    Exp = mybir.ActivationFunctionType.Exp

    nc = bacc.Bacc("TRN2", target_bir_lowering=False, debug=False, num_devices=8)

    qt = nc.dram_tensor("qt", [P, DO, S], F32R, kind="ExternalInput")
    kt = nc.dram_tensor("kt", [P, DO, S], F32R, kind="ExternalInput")
    vt = nc.dram_tensor("vt", [P, DO, S], F32R, kind="ExternalInput")
    wq = nc.dram_tensor("wq", [P, HPC, DO, DK], F32R, kind="ExternalInput")
    wk = nc.dram_tensor("wk", [P, HPC, DO, DK], F32R, kind="ExternalInput")
    wv = nc.dram_tensor("wv", [P, DO, DK], F32R, kind="ExternalInput")
    wo = nc.dram_tensor("wo", [DK, D], F32R, kind="ExternalInput")
    ones = nc.dram_tensor("ones", [P, 1], F32R, kind="ExternalInput")

    attn_t = nc.dram_tensor("attn_t", [HPC, S, S], F32, kind="ExternalOutput")
    out_p = nc.dram_tensor("out_p", [S, D], F32, kind="ExternalOutput")

    with tile.TileContext(nc) as tc:
        with (
            tc.tile_pool(name="inp", bufs=1) as inp,
            tc.tile_pool(name="big", bufs=20) as big,
            tc.tile_pool(name="proj", bufs=2) as proj,
            tc.tile_pool(name="misc", bufs=1) as misc,
            tc.tile_pool(name="rbcp", bufs=2) as rbcp,
            tc.tile_pool(name="outp", bufs=3) as outp,
            tc.tile_pool(name="ps_mm", bufs=2, space="PSUM") as ps_mm,
            tc.tile_pool(name="ps_sc", bufs=2, space="PSUM") as ps_sc,
            tc.tile_pool(name="ps_av", bufs=2, space="PSUM") as ps_av,
            tc.tile_pool(name="ps_dn", bufs=2, space="PSUM") as ps_dn,
        ):
            qt_sb = inp.tile([P, DO, S], F32R)
            kt_sb = inp.tile([P, DO, S], F32R)
            wq_sb = inp.tile([P, HPC, DO, DK], F32R)
            wk_sb = inp.tile([P, HPC, DO, DK], F32R)
            wv_sb = inp.tile([P, DO, DK], F32R)
            wo_sb = inp.tile([DK, D], F32R)
            ones_sb = inp.tile([P, 1], F32R)
            ident = inp.tile([P, P], F32)
            nc.sync.dma_start(qt_sb[:], qt[:])
            nc.sync.dma_start(kt_sb[:], kt[:])
            nc.sync.dma_start(wq_sb[:], wq[:])
            nc.sync.dma_start(wk_sb[:], wk[:])
            nc.sync.dma_start(wv_sb[:], wv[:])
            nc.sync.dma_start(wo_sb[:], wo[:])
            nc.sync.dma_start(ones_sb[:], ones[:])
            make_identity(nc, ident[:])

            # ---- V projection: vsT[k,t] = Wv^T @ v^T, then PE-transpose to vs[t,k]
            vt_ch = {}
            for do in range(DO):
                for c2 in range(SC):
                    t = big.tile([P, N5], F32R, tag="big")
                    nc.sync.dma_start(t[:], vt[:, do, c2 * N5:(c2 + 1) * N5])
                    vt_ch[do, c2] = t
            vsT_sb = misc.tile([P, S], F32)  # [k, t]
            for c2 in range(SC):
                vps = ps_mm.tile([P, N5], F32, tag="ps_mm")
                for do in range(DO):
                    nc.tensor.matmul(vps[:], wv_sb[:, do, :], vt_ch[do, c2][:],
                                     start=(do == 0), stop=(do == DO - 1))
                nc.vector.tensor_copy(vsT_sb[:, c2 * N5:(c2 + 1) * N5], vps[:])
            vs_sb = misc.tile([P, DO, DK], F32R)  # [t, t_outer, k]
            for tt in range(DO):
                trp = ps_sc.tile([P, P], F32, tag="ps_sc")
                nc.tensor.transpose(trp[:], vsT_sb[:, tt * P:(tt + 1) * P], ident[:])
                nc.vector.tensor_copy(vs_sb[:, tt, :], trp[:])

            # ---- head loop
            avp = [ps_av.tile([P, N5], F32, tag="ps_av", name=f"avp{c2}")
                   for c2 in range(SC)]
            for h in range(HPC):
                qsT = proj.tile([P, S], F32R, tag="qsT")  # [k, s]
                ksT = proj.tile([P, S], F32R, tag="ksT")  # [k, t]
                for w_sb, src_sb, dst in ((wq_sb, qt_sb, qsT), (wk_sb, kt_sb, ksT)):
                    for c2 in range(SC):
                        pps = ps_mm.tile([P, N5], F32, tag="ps_mm")
                        for do in range(DO):
                            nc.tensor.matmul(
                                pps[:], w_sb[:, h, do, :],
                                src_sb[:, do, c2 * N5:(c2 + 1) * N5],
                                start=(do == 0), stop=(do == DO - 1))
                        nc.vector.tensor_copy(dst[:, c2 * N5:(c2 + 1) * N5], pps[:])

                # scoresT + exp: expT[t,s] = exp(ksT^T @ qsT)
                expT = {}
                for tt in range(DO):
                    for c2 in range(SC):
                        sps = ps_sc.tile([P, N5], F32, tag="ps_sc")
                        nc.tensor.matmul(sps[:], ksT[:, tt * P:(tt + 1) * P],
                                         qsT[:, c2 * N5:(c2 + 1) * N5],
                                         start=True, stop=True)
                        e = big.tile([P, N5], F32R, tag="big")
                        nc.scalar.activation(out=e[:], in_=sps[:], func=Exp)
                        expT[tt, c2] = e

                # softmax denominator over t (partition dim) via ones-matmul
                recip = misc.tile([1, S], F32, tag="recip")
                for c2 in range(SC):
                    dps = ps_dn.tile([1, N5], F32, tag="ps_dn")
                    for tt in range(DO):
                        nc.tensor.matmul(dps[:], ones_sb[:], expT[tt, c2][:],
                                         start=(tt == 0), stop=(tt == DO - 1))
                    nc.vector.reciprocal(recip[:, c2 * N5:(c2 + 1) * N5], dps[:])
                rbc = rbcp.tile([P, S], F32, tag="rbc")
                nc.gpsimd.partition_broadcast(rbc[:], recip[:])

                # normalize, write attn, accumulate AV over (t, h) into PSUM
                for tt in range(DO):
                    for c2 in range(SC):
                        a = big.tile([P, N5], F32R, tag="big")
                        nc.vector.tensor_mul(a[:], expT[tt, c2][:],
                                             rbc[:, c2 * N5:(c2 + 1) * N5])
                        nc.sync.dma_start(
                            attn_t[h, tt * P:(tt + 1) * P, c2 * N5:(c2 + 1) * N5],
                            a[:].bitcast(F32))
                        nc.tensor.matmul(avp[c2][:], vs_sb[:, tt, :], a[:],
                                         start=(h == 0 and tt == 0),
                                         stop=(h == HPC - 1 and tt == DO - 1),
                                         skip_group_check=True)

            # ---- output projection: out[s,m] = pooledT^T @ (Wo/H)
            pooledT = misc.tile([P, S], F32R)  # [k, s]
            for c2 in range(SC):
                nc.vector.tensor_copy(pooledT[:, c2 * N5:(c2 + 1) * N5], avp[c2][:])
            for st in range(DO):
                for mc in range(SC):
                    ops = ps_mm.tile([P, N5], F32, tag="ps_mm")
                    nc.tensor.matmul(ops[:], pooledT[:, st * P:(st + 1) * P],
                                     wo_sb[:, mc * N5:(mc + 1) * N5],
                                     start=True, stop=True)
                    ot = outp.tile([P, N5], F32, tag="ot")
                    nc.vector.tensor_copy(ot[:], ops[:])
                    nc.sync.dma_start(
                        out_p[st * P:(st + 1) * P, mc * N5:(mc + 1) * N5], ot[:])

    nc.compile()
    return nc


def _pack_inputs(q, k, v, Wq, Wk, Wv, Wo):
    """Build the 8 per-core input maps (host-side shard + layout packing)."""
    f32 = np.float32
    scale = np.float32(DK ** -0.5)
    # [p, h, do, k] = W[h, do*128+p, k]
    wq_g, wk_g = [], []
    for g in range(2):
        wq_g.append(np.ascontiguousarray(
            (Wq[g * HPC:(g + 1) * HPC] * scale).astype(f32)
            .reshape(HPC, DO, P, DK).transpose(2, 0, 1, 3)))
        wk_g.append(np.ascontiguousarray(
            Wk[g * HPC:(g + 1) * HPC].astype(f32)
            .reshape(HPC, DO, P, DK).transpose(2, 0, 1, 3)))
    wv_pack = np.ascontiguousarray(
        Wv.astype(f32).reshape(DO, P, DK).transpose(1, 0, 2))
    wo_pack = np.ascontiguousarray((Wo.astype(f32) / np.float32(H)))
    ones = np.ones([P, 1], f32)

    def pack_t(x):  # [S, D] -> [p, do, s] with d = do*128+p
        return np.ascontiguousarray(
            x.T.reshape(DO, P, S).transpose(1, 0, 2))

    in_maps = []
    for c in range(8):
        b, g = divmod(c, 2)
        in_maps.append({
            "qt": pack_t(np.asarray(q[b], f32)),
            "kt": pack_t(np.asarray(k[b], f32)),
            "vt": pack_t(np.asarray(v[b], f32)),
            "wq": wq_g[g], "wk": wk_g[g], "wv": wv_pack, "wo": wo_pack,
            "ones": ones,
        })
    return in_maps


def kernel(q, k, v, Wq, Wk, Wv, Wo):
    global LAST_RESULTS
    from concourse.bass_utils import run_bass_kernel_spmd

    if "nc" not in _CACHE:
        _CACHE["nc"] = _build()
    nc = _CACHE["nc"]

    in_maps = _pack_inputs(q, k, v, Wq, Wk, Wv, Wo)
    res = run_bass_kernel_spmd(nc, in_maps, core_ids=list(range(8)), trace=TRACE)
    LAST_RESULTS = res

    attn = np.empty((H, B, S, S), np.float32)
    outputs = np.empty((B, S, D), np.float32)
    for c in range(8):
        b, g = divmod(c, 2)
        attn[g * HPC:(g + 1) * HPC, b] = res.results[c]["attn_t"].transpose(0, 2, 1)
    for b in range(B):
        outputs[b] = res.results[2 * b]["out_p"] + res.results[2 * b + 1]["out_p"]
    return outputs, attn
